# revision 1
# baseline (speedup 1.0000x reference)
"""BatchedLoRA trn2 kernel: out[t,n,o] = 2.0 * (x @ A[n].T) @ B[n].T.

Sharding: data-parallel over T across 8 cores (1024 tokens each); every core
computes all 8 experts for its token slab.

Per-core dataflow (all matmuls in fp32r, rounded via DVE copies):
  mm1: adT[r_all=512, t] = A_allT.T-contract-d xT   (lhsT=A_T tiles, rhs=xT tiles)
  mm2: out[t, o] per expert; experts paired (2m at partitions 0-63, 2m+1 at
       64-127) so the two K=64 matmuls occupy disjoint PE row groups.
Host-side prep: transpose x/A/B; fold the 2.0 scaling into B.
"""
import numpy as np
from contextlib import ExitStack

from concourse import bacc, tile, mybir
from concourse.bass_utils import run_bass_kernel_spmd

# Problem dims (hardcoded per contract)
T, D, DO, R, NE = 8192, 2048, 2048, 64, 8
SCALE = 2.0
N_CORES = 8
TC = T // N_CORES          # tokens per core = 1024
P = 128
KT = D // P                # 16 d-tiles
TCH = TC // 512            # 2 t-chunks of 512 (mm1 moving dim)
NP = NE // 2               # 4 expert pairs
TT = TC // P               # 8 t-chunks of 128 (mm2 stationary dim)
OC = DO // 512             # 4 o-chunks of 512 (mm2 moving dim)

F32 = mybir.dt.float32
F32R = mybir.dt.float32r


def build_nc(reps: int = 1):
    """Build the per-core bass program. reps>1 repeats the whole body for
    differential timing."""
    nc = bacc.Bacc("TRN2", target_bir_lowering=False, debug=False)
    xT_ap = nc.dram_tensor("xT", [D, TC], F32, kind="ExternalInput").ap()
    aT_ap = nc.dram_tensor("aT", [D, NE * R], F32, kind="ExternalInput").ap()
    bT_ap = nc.dram_tensor("bT", [NP, P, DO], F32, kind="ExternalInput").ap()
    out_ap = nc.dram_tensor("out", [TC, NE, DO], F32, kind="ExternalOutput").ap()

    with tile.TileContext(nc) as tc, ExitStack() as ctx:
        stage = ctx.enter_context(tc.tile_pool(name="stage", bufs=3))
        xr_p = ctx.enter_context(tc.tile_pool(name="xr", bufs=1))
        ar_p = ctx.enter_context(tc.tile_pool(name="ar", bufs=1))
        br_p = ctx.enter_context(tc.tile_pool(name="br", bufs=1))
        ad_p = ctx.enter_context(tc.tile_pool(name="ad", bufs=1))
        ps1_p = ctx.enter_context(tc.tile_pool(name="ps1", bufs=3, space="PSUM"))
        ps2_p = ctx.enter_context(tc.tile_pool(name="ps2", bufs=4, space="PSUM"))
        ostage = ctx.enter_context(tc.tile_pool(name="ostage", bufs=6))

        xr = xr_p.tile([P, KT * TC], F32R)
        ar = ar_p.tile([P, KT * NE * R], F32R)
        br = br_p.tile([P, NP * DO], F32R)
        ad = ad_p.tile([P, NP * TC], F32R)

        ev = 0
        for _ in range(reps):
            # ---- load + round inputs ----
            for k in range(KT):
                st = stage.tile([P, 2048], F32, tag="st", name=f"stx{k}")
                nc.sync.dma_start(st[:, :TC], xT_ap[k * P:(k + 1) * P, :])
                nc.vector.tensor_copy(xr[:, k * TC:(k + 1) * TC], st[:, :TC])
            for k in range(KT):
                st = stage.tile([P, 2048], F32, tag="st", name=f"sta{k}")
                nc.sync.dma_start(st[:, :NE * R], aT_ap[k * P:(k + 1) * P, :])
                nc.vector.tensor_copy(ar[:, k * NE * R:(k + 1) * NE * R],
                                      st[:, :NE * R])
            for m in range(NP):
                st = stage.tile([P, 2048], F32, tag="st", name=f"stb{m}")
                nc.sync.dma_start(st[:, :DO], bT_ap[m, :, :])
                nc.vector.tensor_copy(br[:, m * DO:(m + 1) * DO], st[:, :DO])

            # ---- per expert pair: mm1 then mm2 ----
            for m in range(NP):
                for tch in range(TCH):
                    ps = ps1_p.tile([P, 512], F32, tag="ps1", name=f"ps1_{m}_{tch}")
                    for k in range(KT):
                        nc.tensor.matmul(
                            ps[:],
                            ar[:, k * NE * R + m * P: k * NE * R + (m + 1) * P],
                            xr[:, k * TC + tch * 512: k * TC + (tch + 1) * 512],
                            start=(k == 0), stop=(k == KT - 1))
                    nc.vector.tensor_copy(
                        ad[:, m * TC + tch * 512: m * TC + (tch + 1) * 512], ps[:])

                for tt in range(TT):
                    for oc in range(OC):
                        for half in range(2):
                            ps = ps2_p.tile([P, 512], F32, tag="ps2",
                                            name=f"ps2_{m}_{tt}_{oc}_{half}")
                            nc.tensor.matmul(
                                ps[:],
                                ad[half * 64:(half + 1) * 64,
                                   m * TC + tt * P: m * TC + (tt + 1) * P],
                                br[half * 64:(half + 1) * 64,
                                   m * DO + oc * 512: m * DO + (oc + 1) * 512],
                                start=True, stop=True)
                            ot = ostage.tile([P, 512], F32, tag="ot",
                                             name=f"ot{ev}")
                            if ev % 2 == 0:
                                nc.vector.tensor_copy(ot[:], ps[:])
                            else:
                                nc.scalar.mul(ot[:], ps[:], 1.0)
                            ev += 1
                            nc.sync.dma_start(
                                out_ap[tt * P:(tt + 1) * P, 2 * m + half,
                                       oc * 512:(oc + 1) * 512],
                                ot[:])
    nc.finalize()
    return nc


def make_in_maps(x, A_weights, B_weights):
    xT = np.ascontiguousarray(x.T)                                  # [D, T]
    aT = np.ascontiguousarray(A_weights.reshape(NE * R, D).T)       # [D, 512]
    b2 = (SCALE * B_weights).transpose(0, 2, 1)                     # [NE, R, DO]
    bT = np.ascontiguousarray(b2.reshape(NP, P, DO))                # pairs stacked
    return [
        {"xT": np.ascontiguousarray(xT[:, c * TC:(c + 1) * TC]),
         "aT": aT, "bT": bT}
        for c in range(N_CORES)
    ]


def kernel(x, A_weights, B_weights):
    x = np.asarray(x, dtype=np.float32)
    A_weights = np.asarray(A_weights, dtype=np.float32)
    B_weights = np.asarray(B_weights, dtype=np.float32)
    nc = build_nc(reps=1)
    in_maps = make_in_maps(x, A_weights, B_weights)
    res = run_bass_kernel_spmd(nc, in_maps, list(range(N_CORES)))
    return np.concatenate([res.results[c]["out"] for c in range(N_CORES)], axis=0)


# revision 4
# speedup vs baseline: 4.4135x; 4.4135x over previous
"""BatchedLoRA trn2 kernel: out[t,n,o] = 2.0 * (x @ A[n].T) @ B[n].T.

Sharding: data-parallel over T across 8 cores (1024 tokens each); every core
computes all 8 experts for its token slab.

Per-core dataflow:
  mm1 (fp32):  adT[r_all=512, t] = contract_d(A_allT, xT)
  mm2 (fp32r): out[t, o] per expert; experts paired (2m at partitions 0-63,
       2m+1 at 64-127) so the two K=64 matmuls occupy disjoint PE row groups.
Host-side prep: transpose x/A/B, fold the 2.0 scale into B, group DRAM layout
into [128, 4096] DMA-friendly blocks.
"""
import numpy as np
from contextlib import ExitStack

from concourse import bacc, tile, mybir
from concourse.bass_utils import run_bass_kernel_spmd

# Problem dims (hardcoded per contract)
T, D, DO, R, NE = 8192, 2048, 2048, 64, 8
SCALE = 2.0
N_CORES = 8
TC = T // N_CORES          # tokens per core = 1024
P = 128
KT = D // P                # 16 d-tiles
TCH = TC // 512            # 2 t-chunks of 512 (mm1 moving dim)
NP = NE // 2               # 4 expert pairs
TT = TC // P               # 8 t-chunks of 128 (mm2 stationary dim)
OC = DO // 512             # 4 o-chunks of 512 (mm2 moving dim)
RA = NE * R                # 512 ranks across experts

F32 = mybir.dt.float32
F32R = mybir.dt.float32r


def build_nc(reps: int = 1, variant: str = "full"):
    """Per-core bass program. reps>1 repeats the body for differential timing.
    variant: full | noout (1/8 of out-DMA) | nomm2 (mm1 only) | inonly."""
    nc = bacc.Bacc("TRN2", target_bir_lowering=False, debug=False)
    x4_ap = nc.dram_tensor("x4", [4, P, 4096], F32, kind="ExternalInput").ap()
    a2_ap = nc.dram_tensor("a2", [2, P, 4096], F32, kind="ExternalInput").ap()
    b2_ap = nc.dram_tensor("b2", [2, P, 4096], F32, kind="ExternalInput").ap()
    out_ap = nc.dram_tensor("out", [TC, NE, DO], F32, kind="ExternalOutput").ap()

    dma_engines = [nc.sync, nc.scalar]

    with tile.TileContext(nc) as tc, ExitStack() as ctx:
        xr_p = ctx.enter_context(tc.tile_pool(name="xr", bufs=1))
        ar_p = ctx.enter_context(tc.tile_pool(name="ar", bufs=1))
        bs_p = ctx.enter_context(tc.tile_pool(name="bs", bufs=1))
        br_p = ctx.enter_context(tc.tile_pool(name="br", bufs=1))
        ad_p = ctx.enter_context(tc.tile_pool(name="ad", bufs=1))
        ps1_p = ctx.enter_context(tc.tile_pool(name="ps1", bufs=2, space="PSUM"))
        ps2_p = ctx.enter_context(tc.tile_pool(name="ps2", bufs=4, space="PSUM"))
        os_p = ctx.enter_context(tc.tile_pool(name="os", bufs=4))

        xr = xr_p.tile([P, KT * TC], F32)        # 64KB/part, fp32 resident
        ar = ar_p.tile([P, KT * RA], F32)        # 32KB/part
        br = br_p.tile([P, NP * DO], F32R)       # 32KB/part
        ad = ad_p.tile([P, NP * TC], F32R)       # 16KB/part

        ev = 0
        for rep in range(reps):
            # ---- input loads (alternate HWDGE rings) ----
            for g in range(4):
                dma_engines[g % 2].dma_start(
                    xr[:, g * 4096:(g + 1) * 4096], x4_ap[g, :, :])
            for g in range(2):
                dma_engines[g % 2].dma_start(
                    ar[:, g * 4096:(g + 1) * 4096], a2_ap[g, :, :])
            for g in range(2):
                bs = bs_p.tile([P, 4096], F32, tag="bs", name=f"bs{rep}_{g}")
                dma_engines[g % 2].dma_start(bs[:], b2_ap[g, :, :])
                nc.vector.tensor_copy(br[:, g * 4096:(g + 1) * 4096], bs[:])

            if variant == "inonly":
                ot = os_p.tile([P, 2048], F32, tag="os", name=f"mark{rep}")
                nc.vector.tensor_copy(ot[:], xr[:, :2048])
                nc.sync.dma_start(out_ap[0:P, 0, :], ot[:])
                continue

            for m in range(NP):
                # ---- mm1 (fp32): adT pair m = [128 r, 1024 t] ----
                for tch in range(TCH):
                    ps = ps1_p.tile([P, 512], F32, tag="ps1",
                                    name=f"ps1_{rep}_{m}_{tch}")
                    for k in range(KT):
                        nc.tensor.matmul(
                            ps[:],
                            ar[:, k * RA + m * P: k * RA + (m + 1) * P],
                            xr[:, k * TC + tch * 512: k * TC + (tch + 1) * 512],
                            start=(k == 0), stop=(k == KT - 1))
                    nc.vector.tensor_copy(
                        ad[:, m * TC + tch * 512: m * TC + (tch + 1) * 512],
                        ps[:])

                if variant == "nomm2":
                    ot = os_p.tile([P, 2048], F32, tag="os",
                                   name=f"mk{rep}_{m}")
                    nc.vector.tensor_copy(
                        ot[:, :TC], ad[:, m * TC:(m + 1) * TC].bitcast(F32))
                    nc.sync.dma_start(out_ap[0:P, m, :], ot[:])
                    continue

                # ---- mm2 (fp32r), experts 2m / 2m+1 ----
                for tt in range(TT):
                    oth = [os_p.tile([P, DO], F32, tag="os",
                                     name=f"os{rep}_{m}_{tt}_{h}")
                           for h in range(2)]
                    for oc in range(OC):
                        for half in range(2):
                            ps = ps2_p.tile([P, 512], F32, tag="ps2",
                                            name=f"ps2_{rep}_{m}_{tt}_{oc}_{half}")
                            nc.tensor.matmul(
                                ps[:],
                                ad[half * 64:(half + 1) * 64,
                                   m * TC + tt * P: m * TC + (tt + 1) * P],
                                br[half * 64:(half + 1) * 64,
                                   m * DO + oc * 512: m * DO + (oc + 1) * 512],
                                start=True, stop=True)
                            dst = oth[half][:, oc * 512:(oc + 1) * 512]
                            if ev % 2 == 0:
                                nc.vector.tensor_copy(dst, ps[:])
                            else:
                                nc.scalar.mul(dst, ps[:], 1.0)
                            ev += 1
                    if variant == "noout" and tt != 0:
                        continue
                    for half in range(2):
                        eng = dma_engines[(tt * NP + m + half) % 2]
                        eng.dma_start(
                            out_ap[tt * P:(tt + 1) * P, 2 * m + half, :],
                            oth[half][:])
    nc.finalize()
    return nc


def make_in_maps(x, A_weights, B_weights):
    xT = np.ascontiguousarray(x.T)                             # [D, T]
    aT = np.ascontiguousarray(A_weights.reshape(RA, D).T)      # [D, 512]
    b2 = (SCALE * B_weights).transpose(0, 2, 1)                # [NE, R, DO]
    bp = b2.reshape(NP, P, DO)                                 # expert pairs

    a2 = aT.reshape(KT, P, RA).transpose(1, 0, 2).reshape(P, KT * RA)
    a2 = np.ascontiguousarray(
        a2.reshape(P, 2, 4096).transpose(1, 0, 2))             # [2, 128, 4096]
    b2g = np.ascontiguousarray(
        bp.reshape(2, 2, P, DO).transpose(0, 2, 1, 3).reshape(2, P, 4096))

    in_maps = []
    for c in range(N_CORES):
        xc = xT[:, c * TC:(c + 1) * TC]                        # [2048, 1024]
        x4 = xc.reshape(KT, P, TC).transpose(1, 0, 2).reshape(P, KT * TC)
        x4 = np.ascontiguousarray(
            x4.reshape(P, 4, 4096).transpose(1, 0, 2))         # [4, 128, 4096]
        in_maps.append({"x4": x4, "a2": a2, "b2": b2g})
    return in_maps


_NC_CACHE = {}


def kernel(x, A_weights, B_weights):
    x = np.asarray(x, dtype=np.float32)
    A_weights = np.asarray(A_weights, dtype=np.float32)
    B_weights = np.asarray(B_weights, dtype=np.float32)
    if "nc" not in _NC_CACHE:
        _NC_CACHE["nc"] = build_nc(reps=1)
    nc = _NC_CACHE["nc"]
    in_maps = make_in_maps(x, A_weights, B_weights)
    res = run_bass_kernel_spmd(nc, in_maps, list(range(N_CORES)))
    return np.concatenate([res.results[c]["out"] for c in range(N_CORES)], axis=0)
